# revision 7
# baseline (speedup 1.0000x reference)
"""Trainium2 Bass kernel for dynamic-scale FP8 GEMM (MixLinear):

    out = (scale_in * scale_w) * (q8(x / scale_in) @ q8(w).T) + bias
    scale_in = max|x| / 448  (global over the whole activation tensor)

Strategy (8 NeuronCores, SPMD):
  - Data-parallel over M = B*S = 16384: each core gets a 2048-row shard of x,
    full weight + bias (replicated).
  - Weight is quantized to fp8 e4m3 ON THE HOST (static scale 1.0 -> plain
    RNE cast; values << 240 so OCP e4m3fn bits == TRN fp8e4 bits) and
    pre-transposed to [K, N].  On-device it is a straight HWDGE load.
  - On-device global amax: per-core abs-max reduce, then an AllGather of the
    8 per-core maxima + local max (cheaper than AllReduce).
  - TRN fp8_e4m3 saturates at +-240 (vs OCP e4m3fn's +-448), so quantize with
    a 2x scale (values land in +-224) and fold the 2x back at dequant time.
  - x is DMA-transposed (fp16, xbar path) into [K-partition, K/128, M] layout
    and quantized on-chip.  The GEMM runs in fp8 DoubleRow perf mode
    (contraction 256 per matmul, free dim 512 = moving-operand cap).
  - Xbar transposes must stay on ONE queue (concurrent transposes corrupt),
    and Tile serializes transpose<->copy transitions globally, so the x
    transposes run as one clean burst ordered before every other DMA.
  - The pre-collective all-engine barrier completes only when every engine
    on every core is idle, i.e. after the transpose burst -- so the
    collective chain (partition_all_reduce -> staging -> doorbell) is kept
    minimal and on otherwise-empty queues.
  - PSUM is evicted with a single ScalarE activation: out = psum*2s + bias
    (output kept N-major: psum partitions = N-tile), so bias is a per-partition
    scalar.  Per-core output is [N, M_shard]; the host transposes on gather.
"""

import os
import sys

try:
    import concourse  # noqa: F401
except ImportError:  # pragma: no cover
    for _p in ("/opt/trn_rl_repo", "/root/.axon_site/_ro/trn_rl_repo"):
        if os.path.isdir(_p) and _p not in sys.path:
            sys.path.insert(0, _p)

import ml_dtypes
import numpy as np

import concourse.bacc as bacc
import concourse.bass as bass  # noqa: F401
import concourse.mybir as mybir
import concourse.tile as tile
from concourse import bass_isa
from concourse.bass_utils import run_bass_kernel_spmd

# Problem shapes (hardcoded per contract).
B, S, K, N = 4, 4096, 2048, 2048
M = B * S
N_CORES = 8
MS = M // N_CORES  # 2048 rows of x per core

P = 128
F16 = mybir.dt.float16
F32 = mybir.dt.float32
FP8 = mybir.dt.float8e4

# x-transpose burst shape: "chunk" = m-chunks of TR_ROWS rows (baseline),
# "plane" = one transpose per k-plane (dst-contiguous).
TR_MODE = os.environ.get("KERNEL_TR_MODE", "plane")
TR_ROWS = int(os.environ.get("KERNEL_TR_ROWS", "512"))


def build_nc(ms=MS, k=K, n=N, n_cores=N_CORES):
    """Build + compile the per-core Bass program (SPMD: same NEFF on all cores)."""
    ko = k // P          # k-outer planes
    assert k % 256 == 0 and ms % 1024 == 0 and n % 256 == 0
    nt_tiles = n // P        # GEMM stationary n-tiles
    k_pairs = ko // 2        # DoubleRow k steps

    nc = bacc.Bacc("TRN2", target_bir_lowering=False, debug=False, num_devices=n_cores)
    x = nc.dram_tensor("x", [ms, k], F16, kind="ExternalInput")
    wq8 = nc.dram_tensor("wq8", [k, n], FP8, kind="ExternalInput")
    b = nc.dram_tensor("b", [n], F16, kind="ExternalInput")
    out_t = nc.dram_tensor("out_t", [n, ms], F16, kind="ExternalOutput")

    with tile.TileContext(nc) as tc:
        with (
            tc.tile_pool(name="big", bufs=1) as big,
            tc.tile_pool(name="small", bufs=1) as small,
            tc.tile_pool(name="ev", bufs=6) as ev,
            tc.tile_pool(name="psum", bufs=2, space="PSUM") as psum,
            tc.tile_pool(name="dram", bufs=1, space="DRAM") as dram,
        ):
            # Persistent SBUF tensors.
            xT = big.tile([P, ko, ms], F16)    # x^T: xT[p, j, m] = x[m, j*128+p]
            xq = big.tile([P, ko, ms], FP8)    # quantized x (scale 2s)
            wq = big.tile([P, ko, n], FP8)     # quantized w (scale 1, host-cast)

            # ---- Phase A ------------------------------------------------
            # Xbar transposes of x: one clean burst on the Sync queue.
            # Abs-max reduces trail each piece, alternating DVE (free-axis
            # XY -> [128,1]) and GpSimd (full XYZWC -> [1,1]; GpSimd only
            # supports partition reductions).
            tr_insts = []
            red_sets = []
            if TR_MODE == "plane":
                pieces = [
                    (x.ap()[:, j * P:(j + 1) * P], xT[:, j]) for j in range(ko)
                ]
            else:
                m_chunks = ms // TR_ROWS
                pieces = []
                for mc in range(m_chunks):
                    lo, hi = mc * TR_ROWS, (mc + 1) * TR_ROWS
                    pieces.append((x.ap()[lo:hi, :], xT[:, :, lo:hi]))
            # GpSimd's XYZWC reduce is ~1.6x slower than DVE's XY, so GpSimd
            # only takes alternate EARLY pieces; every late piece goes to DVE
            # so the post-burst reduce tail is a single fast DVE op.
            n_gps = min(5, len(pieces) // 3)
            gps_idx = {2 * g + 1 for g in range(n_gps)}
            n_dve = len(pieces) - len(gps_idx)
            acc_cols = small.tile([P, n_dve], F32)
            acc_sc = small.tile([P, max(len(gps_idx), 1)], F32)
            nd = ng = 0
            for i, (src_ap, dst_ap) in enumerate(pieces):
                ti = nc.sync.dma_start(out=dst_ap, in_=src_ap, transpose=True)
                tr_insts.append(ti)
                if i in gps_idx:
                    red_sets.append(("gps", acc_sc[0:1, ng:ng + 1], dst_ap))
                    ng += 1
                else:
                    red_sets.append(("dve", acc_cols[:, nd:nd + 1], dst_ap))
                    nd += 1
            for kind, dst, src in red_sets:
                if kind == "dve":
                    nc.vector.tensor_reduce(
                        dst, src,
                        axis=mybir.AxisListType.XY,
                        op=mybir.AluOpType.max,
                        apply_absolute_value=True,
                    )
                else:
                    nc.gpsimd.tensor_reduce(
                        dst, src,
                        axis=mybir.AxisListType.XYZWC,
                        op=mybir.AluOpType.max,
                        apply_absolute_value=True,
                    )
            # Combine: DVE cols -> [128,1]; C-reduce -> [1,1]; max with the
            # GpSimd scalars' X-reduce -> cc staging value.
            amax_col = small.tile([P, 1], F32)
            nc.vector.tensor_reduce(
                amax_col, acc_cols[:], axis=mybir.AxisListType.X,
                op=mybir.AluOpType.max,
            )
            amax_d0 = small.tile([P, 1], F32)
            nc.gpsimd.tensor_reduce(
                amax_d0[0:1, :], amax_col, axis=mybir.AxisListType.C,
                op=mybir.AluOpType.max,
            )
            amax_g0 = small.tile([P, 1], F32)
            nc.vector.tensor_reduce(
                amax_g0[0:1, :], acc_sc[0:1, :], axis=mybir.AxisListType.X,
                op=mybir.AluOpType.max,
            )
            amax_all = small.tile([P, 1], F32)
            nc.vector.scalar_tensor_tensor(
                amax_all[0:1, :], amax_d0[0:1, :], 1.0, amax_g0[0:1, :],
                mybir.AluOpType.mult, mybir.AluOpType.max,
            )

            # ---- AllGather amaxes across cores, reduce locally ------------
            cc_in = dram.tile([1], F32)
            cc_addr = "Shared" if n_cores > 4 else "Local"
            cc_out = dram.tile([n_cores], F32, addr_space=cc_addr)
            cci = nc.scalar.dma_start(cc_in[:], amax_all[0:1, 0])
            tile.add_dep_helper(
                cci.ins, tr_insts[-1].ins,
                reason="xbar: cc staging after transpose burst",
            )
            nc.gpsimd.collective_compute(
                "AllGather",
                mybir.AluOpType.bypass,
                replica_groups=[list(range(n_cores))],
                ins=[cc_in.opt()],
                outs=[cc_out.opt()],
            )
            # ---- Phase W: weight (already fp8) straight HWDGE load --------
            # Ordered after the transpose burst (DMA copies conflict with
            # xbar-transpose mode); chunked along n so the GEMM can start
            # on the first n-range.  k = j*128 + p matches the transpose.
            for i in range(4):
                n0 = i * (n // 4)
                wi = nc.sync.dma_start(
                    out=wq[:, :, n0:n0 + n // 4],
                    in_=wq8.ap()[:, n0:n0 + n // 4].rearrange(
                        "(j p) n2 -> p j n2", p=P
                    ),
                )
                tile.add_dep_helper(
                    wi.ins, tr_insts[-1].ins,
                    reason="xbar: weight copy after transpose burst",
                )

            # bias -> SBUF [128, n/128] fp32, [p, j] = bias[j*128 + p]
            bias16 = small.tile([P, nt_tiles], F16)
            bi = nc.sync.dma_start(
                bias16[:], b.ap().rearrange("(j p) -> p j", p=P)
            )
            tile.add_dep_helper(
                bi.ins, tr_insts[-1].ins,
                reason="xbar: bias copy after transpose burst",
            )
            bias32 = small.tile([P, nt_tiles], F32)
            nc.vector.tensor_copy(bias32[:], bias16[:])

            # Readback; compute inv2s = 224/amax (quant scale) and
            # s2 = amax/224 (dequant scale) on partition 0, then one
            # broadcast of the packed [1,2] pair.
            scal0 = small.tile([P, n_cores], F32)
            nc.scalar.dma_start(scal0[0:1, :], cc_out[:])
            amax1 = small.tile([P, 1], F32)
            nc.vector.tensor_reduce(
                amax1[0:1, :], scal0[0:1, :], axis=mybir.AxisListType.X,
                op=mybir.AluOpType.max,
            )
            sc_pair = small.tile([P, 2], F32)
            inv_amax = small.tile([P, 1], F32)
            nc.vector.reciprocal(inv_amax[0:1, :], amax1[0:1, :])
            nc.vector.tensor_scalar_mul(
                sc_pair[0:1, 0:1], inv_amax[0:1, :], 224.0
            )
            nc.vector.tensor_scalar_mul(
                sc_pair[0:1, 1:2], amax1[0:1, :], 1.0 / 224.0
            )
            sc_bc = small.tile([P, 2], F32)
            nc.gpsimd.partition_broadcast(sc_bc, sc_pair[0:1, :], channels=P)
            inv2s = sc_bc[:, 0:1]
            s2 = sc_bc[:, 1:2]

            # ---- Phases Q+G interleaved: quantize a 512-m quarter, GEMM it.
            # Quantization alternates VectorE (tensor_scalar, ~2x mode) and
            # ScalarE (activation w/ scale) per quarter.
            for mq in range(ms // 512):
                h0 = slice(mq * 512, mq * 512 + 256)
                h1 = slice(mq * 512 + 256, (mq + 1) * 512)
                nc.vector.tensor_scalar(
                    xq[:, :, h0], xT[:, :, h0], inv2s, None,
                    mybir.AluOpType.mult,
                )
                nc.scalar.activation(
                    xq[:, :, h1], xT[:, :, h1],
                    mybir.ActivationFunctionType.Copy, scale=inv2s,
                )
                m0 = mq * 512
                for nt in range(nt_tiles):
                    ps = psum.tile(
                        [P, 512], F32, tag="ps", bufs=7, name=f"ps_{mq}_{nt}"
                    )
                    for k8 in range(k_pairs):
                        nc.tensor.matmul(
                            ps[:],
                            lhsT=wq[:, 2 * k8:2 * k8 + 2, nt * P:(nt + 1) * P],
                            rhs=xq[:, 2 * k8:2 * k8 + 2, m0:m0 + 512],
                            start=(k8 == 0),
                            stop=(k8 == k_pairs - 1),
                            perf_mode=mybir.MatmulPerfMode.DoubleRow,
                        )
                    ob = ev.tile([P, 512], F16, tag="ob", name=f"ob_{mq}_{nt}")
                    nc.scalar.activation(
                        ob[:], ps[:],
                        mybir.ActivationFunctionType.Identity,
                        bias=bias32[:, nt:nt + 1],
                        scale=s2,
                    )
                    nc.sync.dma_start(
                        out_t.ap()[nt * P:(nt + 1) * P, m0:m0 + 512], ob[:]
                    )

    nc.compile()
    return nc


_NC_CACHE = {}


def _get_nc():
    if "nc" not in _NC_CACHE:
        _NC_CACHE["nc"] = build_nc()
    return _NC_CACHE["nc"]


def kernel(x, weight, bias):
    x = np.asarray(x, dtype=np.float16).reshape(M, K)
    weight = np.asarray(weight, dtype=np.float16)
    bias = np.asarray(bias, dtype=np.float16)

    nc = _get_nc()
    # Static-weight host prep: quantize (scale 1.0 -> plain RNE cast to the
    # reference's e4m3fn grid; |w|<240 so bits == TRN fp8e4) and transpose
    # to [K, N].
    wq8 = np.ascontiguousarray(
        weight.astype(np.float32).astype(ml_dtypes.float8_e4m3fn).T
    )
    in_maps = [
        {"x": x[c * MS:(c + 1) * MS], "wq8": wq8, "b": bias}
        for c in range(N_CORES)
    ]
    trace = bool(int(os.environ.get("KERNEL_TRACE", "0")))
    res = run_bass_kernel_spmd(nc, in_maps, list(range(N_CORES)), trace=trace)
    _NC_CACHE["last_result"] = res

    out = np.empty((M, N), dtype=np.float16)
    for c in range(N_CORES):
        out[c * MS:(c + 1) * MS, :] = res.results[c]["out_t"].T
    return out.reshape(B, S, N)


# revision 9
# speedup vs baseline: 1.0143x; 1.0143x over previous
"""Trainium2 Bass kernel for dynamic-scale FP8 GEMM (MixLinear):

    out = (scale_in * scale_w) * (q8(x / scale_in) @ q8(w).T) + bias
    scale_in = max|x| / 448  (global over the whole activation tensor)

Strategy (8 NeuronCores, SPMD, data-parallel over M = B*S = 16384):

  - NeuronCores start a NEFF up to ~30us apart (runtime dispatch skew); the
    amax AllGather is the one rendezvous, so total = skew + pre-collective
    phase + collective + post phase.  v4 minimizes the PRE phase: x is
    loaded in its natural [m-partition, k] layout (fast contiguous DMA,
    ~24us) with abs-max reduces trailing each chunk, instead of the slow
    (~50us) fp16 xbar-transpose burst.
  - Weight is quantized to fp8 e4m3 ON THE HOST (static scale 1.0 -> plain
    RNE cast; values << 240 so OCP e4m3fn bits == TRN fp8e4 bits), packed in
    k-PAIR order (see below), and loaded with a straight HWDGE DMA.
  - TRN fp8_e4m3 saturates at +-240 (vs OCP e4m3fn's +-448), so x is
    quantized with a 2x scale (values land in +-224) and the 2x folds back
    into the dequant scale.
  - After the collective, x is quantized in NATURAL layout (fp8 [m-part, k])
    and then transposed on-chip by viewing adjacent fp8 k-PAIRS as one fp16
    element: a [128m, 1024]-fp16 xbar transpose moves HALF the bytes of a
    fp16 x transpose and lands fp8 pairs contiguously.  The DoubleRow GEMM
    reads the pair with a [128, 2(stride 1), 512(stride 2)] moving AP, and
    the host packs the weight rows in matching k-pair order.
  - These per-m-block transposes run DURING the GEMM.  Xbar transposes stay
    on ONE queue and every output-eviction DMA is ordered after the last
    transpose (transpose||copy hazards); a deep ob pool absorbs the backlog.
  - PSUM is evicted with a single ScalarE activation: out = psum*2s + bias
    (output N-major: psum partitions = n-tile, bias is a per-partition
    scalar).  Per-core output is [N, M_shard]; the host transposes on gather.
"""

import os
import sys

try:
    import concourse  # noqa: F401
except ImportError:  # pragma: no cover
    for _p in ("/opt/trn_rl_repo", "/root/.axon_site/_ro/trn_rl_repo"):
        if os.path.isdir(_p) and _p not in sys.path:
            sys.path.insert(0, _p)

import ml_dtypes
import numpy as np

import concourse.bacc as bacc
import concourse.bass as bass  # noqa: F401
import concourse.mybir as mybir
import concourse.tile as tile
from concourse.bass_utils import run_bass_kernel_spmd

# Problem shapes (hardcoded per contract).
B, S, K, N = 4, 4096, 2048, 2048
M = B * S
N_CORES = 8
MS = M // N_CORES  # 2048 rows of x per core

P = 128
F16 = mybir.dt.float16
F32 = mybir.dt.float32
FP8 = mybir.dt.float8e4


def build_nc(ms=MS, k=K, n=N, n_cores=N_CORES):
    """Build + compile the per-core Bass program (SPMD: same NEFF on all cores)."""
    ko = k // P          # k planes (128 each)
    kj = ko // 2         # DoubleRow k steps (256 each)
    mg_n = ms // P       # m blocks (128 rows each)
    nt_tiles = n // P    # GEMM stationary n-tiles
    assert k % 256 == 0 and ms % 512 == 0 and n % 256 == 0

    nc = bacc.Bacc("TRN2", target_bir_lowering=False, debug=False, num_devices=n_cores)
    x = nc.dram_tensor("x", [ms, k], F16, kind="ExternalInput")
    wq8 = nc.dram_tensor("wq8", [k, n], FP8, kind="ExternalInput")
    b = nc.dram_tensor("b", [n], F16, kind="ExternalInput")
    out_t = nc.dram_tensor("out_t", [n, ms], F16, kind="ExternalOutput")

    with tile.TileContext(nc) as tc:
        with (
            tc.tile_pool(name="big", bufs=1) as big,
            tc.tile_pool(name="small", bufs=1) as small,
            tc.tile_pool(name="ev", bufs=12) as ev,
            tc.tile_pool(name="psum", bufs=2, space="PSUM") as psum,
            tc.tile_pool(name="dram", bufs=1, space="DRAM") as dram,
        ):
            # Persistent SBUF tensors.
            xnat = big.tile([P, mg_n, k], F16)   # x natural: [p, mg, k] = x[mg*128+p, k]
            xqn = big.tile([P, mg_n, k], FP8)    # quantized x, natural layout
            # packed transpose target: fp16 element [q, jj, m] = fp8 pair
            # (k = 2*(jj*128+q) + {0,1}) of column m
            xqT = big.tile([P, kj, ms], F16)
            wq = big.tile([P, ko, n], FP8)       # w fp8, host k-pair packing

            # ---- Phase A: natural x load + amax -------------------------
            # 16 contiguous chunk loads on the Sync queue; abs-max reduces
            # trail each chunk.  GpSimd's XYZWC reduce is ~1.6x slower than
            # DVE's XY reduce, so GpSimd only takes alternate EARLY chunks
            # and every late chunk goes to DVE (single fast tail).
            n_gps = min(6, mg_n // 3)
            gps_idx = {2 * g + 1 for g in range(n_gps)}
            n_dve = mg_n - len(gps_idx)
            acc_cols = small.tile([P, n_dve], F32)
            acc_sc = small.tile([P, max(len(gps_idx), 1)], F32)
            nd = ng = 0
            for mg in range(mg_n):
                nc.sync.dma_start(
                    out=xnat[:, mg], in_=x.ap()[mg * P:(mg + 1) * P, :]
                )
                if mg in gps_idx:
                    nc.gpsimd.tensor_reduce(
                        acc_sc[0:1, ng:ng + 1], xnat[:, mg],
                        axis=mybir.AxisListType.XYZWC,
                        op=mybir.AluOpType.max,
                        apply_absolute_value=True,
                    )
                    ng += 1
                else:
                    nc.vector.tensor_reduce(
                        acc_cols[:, nd:nd + 1], xnat[:, mg],
                        axis=mybir.AxisListType.XY,
                        op=mybir.AluOpType.max,
                        apply_absolute_value=True,
                    )
                    nd += 1
            # Combine partial maxima -> one [1,1] scalar on partition 0.
            amax_col = small.tile([P, 1], F32)
            nc.vector.tensor_reduce(
                amax_col, acc_cols[:], axis=mybir.AxisListType.X,
                op=mybir.AluOpType.max,
            )
            amax_d0 = small.tile([P, 1], F32)
            nc.gpsimd.tensor_reduce(
                amax_d0[0:1, :], amax_col, axis=mybir.AxisListType.C,
                op=mybir.AluOpType.max,
            )
            amax_g0 = small.tile([P, 1], F32)
            nc.vector.tensor_reduce(
                amax_g0[0:1, :], acc_sc[0:1, :], axis=mybir.AxisListType.X,
                op=mybir.AluOpType.max,
            )
            amax_all = small.tile([P, 1], F32)
            nc.vector.scalar_tensor_tensor(
                amax_all[0:1, :], amax_d0[0:1, :], 1.0, amax_g0[0:1, :],
                mybir.AluOpType.mult, mybir.AluOpType.max,
            )

            # ---- Weight + bias loads (Scalar HWDGE queue, overlap phase A)
            for i in range(4):
                n0 = i * (n // 4)
                nc.scalar.dma_start(
                    out=wq[:, :, n0:n0 + n // 4],
                    in_=wq8.ap()[:, n0:n0 + n // 4].rearrange(
                        "(j p) n2 -> p j n2", p=P
                    ),
                )
            bias16 = small.tile([P, nt_tiles], F16)
            nc.scalar.dma_start(
                bias16[:], b.ap().rearrange("(j p) -> p j", p=P)
            )
            bias32 = small.tile([P, nt_tiles], F32)
            nc.vector.tensor_copy(bias32[:], bias16[:])

            # ---- AllGather amaxes across cores, reduce locally ------------
            cc_in = dram.tile([1], F32)
            cc_addr = "Shared" if n_cores > 4 else "Local"
            cc_out = dram.tile([n_cores], F32, addr_space=cc_addr)
            nc.scalar.dma_start(cc_in[:], amax_all[0:1, 0])
            nc.gpsimd.collective_compute(
                "AllGather",
                mybir.AluOpType.bypass,
                replica_groups=[list(range(n_cores))],
                ins=[cc_in.opt()],
                outs=[cc_out.opt()],
            )
            # Readback; inv2s = 224/amax (quant scale), s2 = amax/224
            # (dequant scale) computed on partition 0, then one broadcast
            # of the packed [1,2] pair.
            scal0 = small.tile([P, n_cores], F32)
            nc.scalar.dma_start(scal0[0:1, :], cc_out[:])
            amax1 = small.tile([P, 1], F32)
            nc.vector.tensor_reduce(
                amax1[0:1, :], scal0[0:1, :], axis=mybir.AxisListType.X,
                op=mybir.AluOpType.max,
            )
            sc_pair = small.tile([P, 2], F32)
            inv_amax = small.tile([P, 1], F32)
            nc.vector.reciprocal(inv_amax[0:1, :], amax1[0:1, :])
            nc.vector.tensor_scalar_mul(
                sc_pair[0:1, 0:1], inv_amax[0:1, :], 224.0
            )
            nc.vector.tensor_scalar_mul(
                sc_pair[0:1, 1:2], amax1[0:1, :], 1.0 / 224.0
            )
            sc_bc = small.tile([P, 2], F32)
            nc.gpsimd.partition_broadcast(sc_bc, sc_pair[0:1, :], channels=P)
            inv2s = sc_bc[:, 0:1]
            s2 = sc_bc[:, 1:2]

            # ---- Quantize (natural layout) + packed transposes + GEMM ----
            # Per m-block: quantize [128, 2048] (first 4 blocks split
            # DVE||ScalarE for the fastest GEMM start; later blocks
            # alternate engines), then one packed-pair xbar transpose
            # (fp16 view of the fp8 pairs, SBUF->SBUF, Sync queue).
            tr_insts = []

            def emit_block(mg):
                if mg < 4:
                    h = k // 2
                    nc.vector.tensor_scalar(
                        xqn[:, mg, 0:h], xnat[:, mg, 0:h], inv2s, None,
                        mybir.AluOpType.mult,
                    )
                    nc.scalar.activation(
                        xqn[:, mg, h:k], xnat[:, mg, h:k],
                        mybir.ActivationFunctionType.Copy, scale=inv2s,
                    )
                elif mg % 2 == 0:
                    nc.vector.tensor_scalar(
                        xqn[:, mg, :], xnat[:, mg, :], inv2s, None,
                        mybir.AluOpType.mult,
                    )
                else:
                    nc.scalar.activation(
                        xqn[:, mg, :], xnat[:, mg, :],
                        mybir.ActivationFunctionType.Copy, scale=inv2s,
                    )
                ti = nc.sync.dma_start(
                    out=xqT[:, :, mg * P:(mg + 1) * P],
                    in_=xqn[:, mg, :].bitcast(F16),
                    transpose=True,
                )
                tr_insts.append(ti)

            # All quant+transpose blocks first in program order so the Sync
            # queue runs loads -> transposes -> evictions (clean transpose
            # burst, no copy interleave); the scheduler pipelines the GEMM
            # in by data deps.
            for mg in range(mg_n):
                emit_block(mg)

            out_dmas = []
            for mq in range(ms // 512):
                m0 = mq * 512
                for nt in range(nt_tiles):
                    ps = psum.tile(
                        [P, 512], F32, tag="ps", bufs=7, name=f"ps_{mq}_{nt}"
                    )
                    for jj in range(kj):
                        rhs = (
                            xqT[:, jj, m0:m0 + 512]
                            .bitcast(FP8)
                            .rearrange("p (m two) -> p two m", two=2)
                        )
                        nc.tensor.matmul(
                            ps[:],
                            lhsT=wq[:, 2 * jj:2 * jj + 2, nt * P:(nt + 1) * P],
                            rhs=rhs,
                            start=(jj == 0),
                            stop=(jj == kj - 1),
                            perf_mode=mybir.MatmulPerfMode.DoubleRow,
                        )
                    ob = ev.tile([P, 512], F16, tag="ob", name=f"ob_{mq}_{nt}")
                    nc.scalar.activation(
                        ob[:], ps[:],
                        mybir.ActivationFunctionType.Identity,
                        bias=bias32[:, nt:nt + 1],
                        scale=s2,
                    )
                    oi = nc.sync.dma_start(
                        out_t.ap()[nt * P:(nt + 1) * P, m0:m0 + 512], ob[:]
                    )
                    out_dmas.append(oi)

            # Order every output DMA after the final transpose (xbar
            # transpose || copy hazard); the ob pool absorbs the backlog.
            for oi in out_dmas[:16]:
                tile.add_dep_helper(
                    oi.ins, tr_insts[-1].ins,
                    reason="xbar: evictions after transpose burst",
                )

    nc.compile()
    return nc


_NC_CACHE = {}


def _get_nc():
    if "nc" not in _NC_CACHE:
        _NC_CACHE["nc"] = build_nc()
    return _NC_CACHE["nc"]


def kernel(x, weight, bias):
    x = np.asarray(x, dtype=np.float16).reshape(M, K)
    weight = np.asarray(weight, dtype=np.float16)
    bias = np.asarray(bias, dtype=np.float16)

    nc = _get_nc()
    # Static-weight host prep: quantize (scale 1.0 -> plain RNE cast onto
    # the reference's e4m3fn grid; |w|<240 so bits == TRN fp8e4), transpose
    # to [K, N], and pack rows in k-PAIR order to match the on-chip packed
    # transpose: DRAM row (jj*256 + pr*128 + q) holds k = jj*256 + 2q + pr.
    w8T = weight.astype(np.float32).astype(ml_dtypes.float8_e4m3fn).T
    wq8 = np.ascontiguousarray(
        w8T.reshape(K // 256, 128, 2, N).transpose(0, 2, 1, 3).reshape(K, N)
    )
    in_maps = [
        {"x": x[c * MS:(c + 1) * MS], "wq8": wq8, "b": bias}
        for c in range(N_CORES)
    ]
    trace = bool(int(os.environ.get("KERNEL_TRACE", "0")))
    res = run_bass_kernel_spmd(nc, in_maps, list(range(N_CORES)), trace=trace)
    _NC_CACHE["last_result"] = res

    out = np.empty((M, N), dtype=np.float16)
    for c in range(N_CORES):
        out[c * MS:(c + 1) * MS, :] = res.results[c]["out_t"].T
    return out.reshape(B, S, N)


# revision 10
# speedup vs baseline: 1.1114x; 1.0958x over previous
"""Trainium2 Bass kernel for dynamic-scale FP8 GEMM (MixLinear):

    out = (scale_in * scale_w) * (q8(x / scale_in) @ q8(w).T) + bias
    scale_in = max|x| / 448  (global over the whole activation tensor)

Strategy (8 NeuronCores, SPMD, data-parallel over M = B*S = 16384):

  - NeuronCores start a NEFF up to ~30us apart (runtime dispatch skew); the
    amax AllGather is the one rendezvous, so total = skew + pre-collective
    phase + collective + post phase.  The PRE phase is minimized: x is
    loaded in its natural [m-partition, k] layout as 8 x 1MB contiguous
    DMAs split across the two HWDGE queues (Sync+Scalar), with abs-max
    reduces (DVE XY + GpSimd XYZWC) trailing each piece.  Weight and bias
    loads wait until after the collective doorbell so they don't steal HBM
    bandwidth from the x read.
  - An optional dummy 4-byte AllGather at program start pre-warms the CC
    stream (the real collective otherwise pays ~11.5us trigger-to-start).
  - Weight is quantized to fp8 e4m3 ON THE HOST (static scale 1.0 -> plain
    RNE cast; values << 240 so OCP e4m3fn bits == TRN fp8e4 bits), packed
    in k-PAIR order, and loaded with a straight HWDGE DMA.
  - TRN fp8_e4m3 saturates at +-240 (vs OCP e4m3fn's +-448), so x is
    quantized with a 2x scale (values land in +-224) and the 2x folds back
    into the dequant scale.
  - After the collective, x is quantized in NATURAL layout (fp8 [m-part,k])
    and transposed on-chip by viewing adjacent fp8 k-PAIRS as one fp16
    element: a [128m, 1024]-fp16 xbar transpose moves HALF the bytes of an
    fp16 transpose and lands fp8 pairs contiguously.  The DoubleRow GEMM
    reads the pair with a [128, 2(stride 1), 512(stride 2)] moving AP, and
    the host packs the weight rows in matching k-pair order.
  - The per-m-block transposes run DURING the GEMM (they feed it block by
    block); the first two GEMM chunks are 256-m so compute starts after
    only 2 blocks.  Xbar transposes stay on ONE queue and output-eviction
    DMAs are queue-ordered after the last transpose (transpose||copy
    hazard); a deep ob pool absorbs the backlog.
  - PSUM is evicted with a single ScalarE activation: out = psum*2s + bias
    (output N-major: psum partitions = n-tile, bias is a per-partition
    scalar).  Per-core output is [N, M_shard]; the host transposes on
    gather.
"""

import os
import sys

try:
    import concourse  # noqa: F401
except ImportError:  # pragma: no cover
    for _p in ("/opt/trn_rl_repo", "/root/.axon_site/_ro/trn_rl_repo"):
        if os.path.isdir(_p) and _p not in sys.path:
            sys.path.insert(0, _p)

import ml_dtypes
import numpy as np

import concourse.bacc as bacc
import concourse.bass as bass  # noqa: F401
import concourse.mybir as mybir
import concourse.tile as tile
from concourse.bass_utils import run_bass_kernel_spmd

# Problem shapes (hardcoded per contract).
B, S, K, N = 4, 4096, 2048, 2048
M = B * S
N_CORES = 8
MS = M // N_CORES  # 2048 rows of x per core

P = 128
F16 = mybir.dt.float16
F32 = mybir.dt.float32
FP8 = mybir.dt.float8e4

WARM_CC = bool(int(os.environ.get("KERNEL_WARMCC", "1")))
# m-block spans (in 128-row blocks) of the GEMM chunks: small lead-in
# chunks so the first matmuls only wait for 2 quant+transpose blocks.
CHUNK_PLAN = [(0, 2), (2, 4), (4, 8), (8, 12), (12, 16)]


def build_nc(ms=MS, k=K, n=N, n_cores=N_CORES):
    """Build + compile the per-core Bass program (SPMD: same NEFF on all cores)."""
    ko = k // P          # k planes (128 each)
    kj = ko // 2         # DoubleRow k steps (256 each)
    mg_n = ms // P       # m blocks (128 rows each)
    nt_tiles = n // P    # GEMM stationary n-tiles
    assert k % 256 == 0 and ms % 512 == 0 and n % 256 == 0
    assert CHUNK_PLAN[-1][1] == mg_n

    nc = bacc.Bacc("TRN2", target_bir_lowering=False, debug=False, num_devices=n_cores)
    x = nc.dram_tensor("x", [ms, k], F16, kind="ExternalInput")
    wq8 = nc.dram_tensor("wq8", [k, n], FP8, kind="ExternalInput")
    b = nc.dram_tensor("b", [n], F16, kind="ExternalInput")
    out_t = nc.dram_tensor("out_t", [n, ms], F16, kind="ExternalOutput")

    with tile.TileContext(nc) as tc:
        with (
            tc.tile_pool(name="big", bufs=1) as big,
            tc.tile_pool(name="small", bufs=1) as small,
            tc.tile_pool(name="ev", bufs=12) as ev,
            tc.tile_pool(name="psum", bufs=2, space="PSUM") as psum,
            tc.tile_pool(name="dram", bufs=1, space="DRAM") as dram,
        ):
            # Persistent SBUF tensors.
            xnat = big.tile([P, mg_n, k], F16)   # x natural: [p, mg, k] = x[mg*128+p, k]
            xqn = big.tile([P, mg_n, k], FP8)    # quantized x, natural layout
            # packed transpose target: fp16 element [q, jj, m] = fp8 pair
            # (k = 2*(jj*128+q) + {0,1}) of column m
            xqT = big.tile([P, kj, ms], F16)
            wq = big.tile([P, ko, n], FP8)       # w fp8, host k-pair packing

            cc_addr = "Shared" if n_cores > 4 else "Local"
            if WARM_CC:
                # Pre-warm the CC stream: a dummy 4-byte AllGather issued at
                # t~0 pays the collective wakeup cost while the x load runs.
                warm_src = small.tile([P, 1], F32)
                nc.gpsimd.memset(warm_src[0:1, :], 0.0)
                warm_in = dram.tile([1], F32)
                warm_out = dram.tile([n_cores], F32, addr_space=cc_addr)
                nc.scalar.dma_start(warm_in[:], warm_src[0:1, 0])
                nc.gpsimd.collective_compute(
                    "AllGather",
                    mybir.AluOpType.bypass,
                    replica_groups=[list(range(n_cores))],
                    ins=[warm_in.opt()],
                    outs=[warm_out.opt()],
                )

            # ---- Phase A: natural x load + amax -------------------------
            # 8 x 1MB contiguous loads (2 m-blocks each), alternating
            # Sync/Scalar HWDGE queues; abs-max reduces trail each piece.
            # GpSimd's XYZWC reduce is ~1.6x slower than DVE's XY reduce,
            # so GpSimd takes 3 early pieces, DVE the rest (fast tail).
            n_ld = mg_n // 2
            gps_idx = {1, 3, 5}
            n_dve = n_ld - len(gps_idx)
            acc_cols = small.tile([P, n_dve], F32)
            acc_sc = small.tile([P, len(gps_idx)], F32)
            nd = ng = 0
            for g in range(n_ld):
                eng = nc.sync if g % 2 == 0 else nc.scalar
                eng.dma_start(
                    out=xnat[:, 2 * g:2 * g + 2, :],
                    in_=x.ap()[g * 256:(g + 1) * 256, :].rearrange(
                        "(b p) k2 -> p b k2", b=2
                    ),
                )
                if g in gps_idx:
                    nc.gpsimd.tensor_reduce(
                        acc_sc[0:1, ng:ng + 1], xnat[:, 2 * g:2 * g + 2, :],
                        axis=mybir.AxisListType.XYZWC,
                        op=mybir.AluOpType.max,
                        apply_absolute_value=True,
                    )
                    ng += 1
                else:
                    nc.vector.tensor_reduce(
                        acc_cols[:, nd:nd + 1], xnat[:, 2 * g:2 * g + 2, :],
                        axis=mybir.AxisListType.XY,
                        op=mybir.AluOpType.max,
                        apply_absolute_value=True,
                    )
                    nd += 1
            # Combine partial maxima -> one [1,1] scalar on partition 0.
            amax_col = small.tile([P, 1], F32)
            nc.vector.tensor_reduce(
                amax_col, acc_cols[:], axis=mybir.AxisListType.X,
                op=mybir.AluOpType.max,
            )
            amax_d0 = small.tile([P, 1], F32)
            nc.gpsimd.tensor_reduce(
                amax_d0[0:1, :], amax_col, axis=mybir.AxisListType.C,
                op=mybir.AluOpType.max,
            )
            amax_g0 = small.tile([P, 1], F32)
            nc.vector.tensor_reduce(
                amax_g0[0:1, :], acc_sc[0:1, :], axis=mybir.AxisListType.X,
                op=mybir.AluOpType.max,
            )
            amax_all = small.tile([P, 1], F32)
            nc.vector.scalar_tensor_tensor(
                amax_all[0:1, :], amax_d0[0:1, :], 1.0, amax_g0[0:1, :],
                mybir.AluOpType.mult, mybir.AluOpType.max,
            )

            # ---- AllGather amaxes across cores, reduce locally ------------
            cc_in = dram.tile([1], F32)
            cc_out = dram.tile([n_cores], F32, addr_space=cc_addr)
            nc.scalar.dma_start(cc_in[:], amax_all[0:1, 0])
            nc.gpsimd.collective_compute(
                "AllGather",
                mybir.AluOpType.bypass,
                replica_groups=[list(range(n_cores))],
                ins=[cc_in.opt()],
                outs=[cc_out.opt()],
            )

            # ---- Weight + bias loads (Scalar queue, after the doorbell so
            # they don't contend with the x read; hidden in the CC window).
            for i in range(2):
                n0 = i * (n // 2)
                nc.scalar.dma_start(
                    out=wq[:, :, n0:n0 + n // 2],
                    in_=wq8.ap()[:, n0:n0 + n // 2].rearrange(
                        "(j p) n2 -> p j n2", p=P
                    ),
                )
            bias16 = small.tile([P, nt_tiles], F16)
            nc.scalar.dma_start(
                bias16[:], b.ap().rearrange("(j p) -> p j", p=P)
            )
            bias32 = small.tile([P, nt_tiles], F32)
            nc.vector.tensor_copy(bias32[:], bias16[:])

            # Readback; inv2s = 224/amax (quant scale), s2 = amax/224
            # (dequant scale) computed on partition 0, then one broadcast
            # of the packed [1,2] pair.
            scal0 = small.tile([P, n_cores], F32)
            nc.scalar.dma_start(scal0[0:1, :], cc_out[:])
            amax1 = small.tile([P, 1], F32)
            nc.vector.tensor_reduce(
                amax1[0:1, :], scal0[0:1, :], axis=mybir.AxisListType.X,
                op=mybir.AluOpType.max,
            )
            sc_pair = small.tile([P, 2], F32)
            inv_amax = small.tile([P, 1], F32)
            nc.vector.reciprocal(inv_amax[0:1, :], amax1[0:1, :])
            nc.vector.tensor_scalar_mul(
                sc_pair[0:1, 0:1], inv_amax[0:1, :], 224.0
            )
            nc.vector.tensor_scalar_mul(
                sc_pair[0:1, 1:2], amax1[0:1, :], 1.0 / 224.0
            )
            sc_bc = small.tile([P, 2], F32)
            nc.gpsimd.partition_broadcast(sc_bc, sc_pair[0:1, :], channels=P)
            inv2s = sc_bc[:, 0:1]
            s2 = sc_bc[:, 1:2]

            # ---- Quantize (natural layout) + packed transposes -----------
            # First 4 blocks split DVE||ScalarE for the fastest GEMM start;
            # later blocks mostly DVE (ScalarE is busy with evictions).
            tr_insts = []

            def emit_block(mg):
                if mg < 4:
                    h = k // 2
                    nc.vector.tensor_scalar(
                        xqn[:, mg, 0:h], xnat[:, mg, 0:h], inv2s, None,
                        mybir.AluOpType.mult,
                    )
                    nc.scalar.activation(
                        xqn[:, mg, h:k], xnat[:, mg, h:k],
                        mybir.ActivationFunctionType.Copy, scale=inv2s,
                    )
                elif mg in (5, 9, 13):
                    nc.scalar.activation(
                        xqn[:, mg, :], xnat[:, mg, :],
                        mybir.ActivationFunctionType.Copy, scale=inv2s,
                    )
                else:
                    nc.vector.tensor_scalar(
                        xqn[:, mg, :], xnat[:, mg, :], inv2s, None,
                        mybir.AluOpType.mult,
                    )
                ti = nc.sync.dma_start(
                    out=xqT[:, :, mg * P:(mg + 1) * P],
                    in_=xqn[:, mg, :].bitcast(F16),
                    transpose=True,
                )
                tr_insts.append(ti)

            # All quant+transpose blocks first in program order so the Sync
            # queue runs loads -> transposes -> evictions (clean transpose
            # burst, no copy interleave); the scheduler pipelines the GEMM
            # in by data deps.
            for mg in range(mg_n):
                emit_block(mg)

            # ---- GEMM (fp8 DoubleRow) + fused eviction -------------------
            out_dmas = []
            for ci, (b0, b1) in enumerate(CHUNK_PLAN):
                m0 = b0 * P
                msz = (b1 - b0) * P
                for nt in range(nt_tiles):
                    ps = psum.tile(
                        [P, msz], F32, tag="ps", bufs=7, name=f"ps_{ci}_{nt}"
                    )
                    for jj in range(kj):
                        rhs = (
                            xqT[:, jj, m0:m0 + msz]
                            .bitcast(FP8)
                            .rearrange("p (m two) -> p two m", two=2)
                        )
                        nc.tensor.matmul(
                            ps[:],
                            lhsT=wq[:, 2 * jj:2 * jj + 2, nt * P:(nt + 1) * P],
                            rhs=rhs,
                            start=(jj == 0),
                            stop=(jj == kj - 1),
                            perf_mode=mybir.MatmulPerfMode.DoubleRow,
                        )
                    ob = ev.tile([P, msz], F16, tag="ob", name=f"ob_{ci}_{nt}")
                    nc.scalar.activation(
                        ob[:], ps[:],
                        mybir.ActivationFunctionType.Identity,
                        bias=bias32[:, nt:nt + 1],
                        scale=s2,
                    )
                    oi = nc.sync.dma_start(
                        out_t.ap()[nt * P:(nt + 1) * P, m0:m0 + msz], ob[:]
                    )
                    out_dmas.append(oi)

            # Order the early output DMAs after the final transpose (xbar
            # transpose || copy hazard); the ob pool absorbs the backlog.
            for oi in out_dmas[:16]:
                tile.add_dep_helper(
                    oi.ins, tr_insts[-1].ins,
                    reason="xbar: evictions after transpose burst",
                )

    nc.compile()
    return nc


_NC_CACHE = {}


def _get_nc():
    if "nc" not in _NC_CACHE:
        _NC_CACHE["nc"] = build_nc()
    return _NC_CACHE["nc"]


def kernel(x, weight, bias):
    x = np.asarray(x, dtype=np.float16).reshape(M, K)
    weight = np.asarray(weight, dtype=np.float16)
    bias = np.asarray(bias, dtype=np.float16)

    nc = _get_nc()
    # Static-weight host prep: quantize (scale 1.0 -> plain RNE cast onto
    # the reference's e4m3fn grid; |w|<240 so bits == TRN fp8e4), transpose
    # to [K, N], and pack rows in k-PAIR order to match the on-chip packed
    # transpose: DRAM row (jj*256 + pr*128 + q) holds k = jj*256 + 2q + pr.
    w8T = weight.astype(np.float32).astype(ml_dtypes.float8_e4m3fn).T
    wq8 = np.ascontiguousarray(
        w8T.reshape(K // 256, 128, 2, N).transpose(0, 2, 1, 3).reshape(K, N)
    )
    in_maps = [
        {"x": x[c * MS:(c + 1) * MS], "wq8": wq8, "b": bias}
        for c in range(N_CORES)
    ]
    trace = bool(int(os.environ.get("KERNEL_TRACE", "0")))
    res = run_bass_kernel_spmd(nc, in_maps, list(range(N_CORES)), trace=trace)
    _NC_CACHE["last_result"] = res

    out = np.empty((M, N), dtype=np.float16)
    for c in range(N_CORES):
        out[c * MS:(c + 1) * MS, :] = res.results[c]["out_t"].T
    return out.reshape(B, S, N)
